# revision 20
# baseline (speedup 1.0000x reference)
"""Trainium2 Bass kernel for nn_CumulativeShadeRegressor.

Model (per sample): per-leaf MLP encoder [L, FD] -> [L, H2] (two gelu
layers), softplus absorb/atten heads, a top-to-bottom exponential
transmittance scan over L, mean-pooling over L, and a small dense head on
[Xg | pooled].

Strategy: data-parallel over B across 8 NeuronCores (32 samples/core).
The ACT (scalar) engine is the bottleneck: 64 gelu ACTIVATEs of 2048 cols
each (~2us apiece) form a ~126us stream that everything else must hide
under.  The kernel therefore:
  * layer 1 (K=64) runs as row-tiled bf16 matmul pairs (2 concurrent MMs
    in disjoint 64-row PE strips);
  * layer 2 runs in fp8e4 DoubleRow mode (2 MACs/cell, contraction 256
    per pass) with h1 quantized to fp8 by the gelu ACT itself;
  * gelu ACT ops are batched to N=2048 (4 samples per instruction, PSUM
    pair ring of 2x4 banks) to amortize the ~300-cycle ACT init;
  * startup DMAs are spread across all three DGE queues (sync, scalar,
    gpsimd) so the first x2 block + w1s land ~3us earlier;
  * PE warm-up/filler matmuls keep the HAM clock gate at K=8/8 through
    the l1-only prologue (a K=4/8 dip used to stall the ACT stream 4.6us);
  * phase 2 gives each absorb/atten burst its own PSUM bank (no ping-pong
    chains), drains 4 bursts on the scalar engine and 4 on the DVE, and
    runs the softplus/scan/transmittance tail in two pipelined halves
    (samples 0-15 and 16-31) so scalar, DVE, and DMA latency overlap;
  * the final [32,1] result is transposed to one SBUF row so the output
    DMA is a single descriptor (the 32-descriptor version left its
    completion semaphore trickling for ~5us after the data landed).
"""
import sys

sys.path.insert(0, "/opt/trn_rl_repo")

import numpy as np
import ml_dtypes

import concourse.bacc as bacc
import concourse.mybir as mybir
import concourse.tile as tile
from concourse.bass_utils import run_bass_kernel_spmd
from concourse.tile import add_dep_helper

B, L, FD, G = 256, 512, 64, 32
H1, H2, DH = 512, 512, 256
NCORES = 8
BL = B // NCORES          # 32 samples per core
NBLK = BL // 4            # 8 blocks of 4 samples

f32 = mybir.dt.float32
bf16 = mybir.dt.bfloat16
f8e4 = mybir.dt.float8e4
AF = mybir.ActivationFunctionType
ALU = mybir.AluOpType
AX = mybir.AxisListType
DR = mybir.MatmulPerfMode.DoubleRow


def _build():
    nc = bacc.Bacc("TRN2", target_bir_lowering=False, debug=False,
                   num_devices=NCORES)

    d = {}
    d["xlt"] = nc.dram_tensor("xlt", [128, NBLK * 1024], bf16, kind="ExternalInput").ap()
    d["xgt"] = nc.dram_tensor("xgt", [G, BL], f32, kind="ExternalInput").ap()
    d["w1s"] = nc.dram_tensor("w1s", [128, H1], bf16, kind="ExternalInput").ap()
    d["w2f8"] = nc.dram_tensor("w2f8", [128, 2048], f8e4, kind="ExternalInput").ap()
    d["wawt"] = nc.dram_tensor("wawt", [128, 8], bf16, kind="ExternalInput").ap()
    d["wd1g"] = nc.dram_tensor("wd1g", [G, DH], f32, kind="ExternalInput").ap()
    d["wd1p"] = nc.dram_tensor("wd1p", [128, 4 * DH], f32, kind="ExternalInput").ap()
    d["wd2"] = nc.dram_tensor("wd2", [128, 2], f32, kind="ExternalInput").ap()
    d["b1"] = nc.dram_tensor("b1", [128, 4], f32, kind="ExternalInput").ap()
    d["b2"] = nc.dram_tensor("b2", [128, 4], f32, kind="ExternalInput").ap()
    d["bd1"] = nc.dram_tensor("bd1", [128, 2], f32, kind="ExternalInput").ap()
    d["scal"] = nc.dram_tensor("scal", [128, 4], f32, kind="ExternalInput").ap()
    out_d = nc.dram_tensor("out", [2, BL // 2], f32, kind="ExternalOutput").ap()

    with tile.TileContext(nc) as tc:
        with (
            tc.tile_pool(name="wp", bufs=1) as wp,
            tc.tile_pool(name="pp", bufs=1) as pp,
            tc.tile_pool(name="xp", bufs=4) as xp,
            tc.tile_pool(name="h1p", bufs=2) as h1p,
            tc.tile_pool(name="p2sb", bufs=1) as p2sb,
            tc.tile_pool(name="awp", bufs=4) as awp,
            tc.tile_pool(name="psp", bufs=1, space="PSUM") as psp,
        ):
            w1s_t = wp.tile([128, H1], bf16)
            w2f8_t = wp.tile([128, 2048], f8e4)
            wawt_t = wp.tile([128, 8], bf16)
            xgt_t = wp.tile([G, BL], f32)
            wd1g_t = wp.tile([G, DH], f32)
            wd1p_t = wp.tile([128, 4 * DH], f32)
            wd2_t = wp.tile([128, 2], f32)
            b1_t = wp.tile([128, 4], f32)
            b2_t = wp.tile([128, 4], f32)
            bd1_t = wp.tile([128, 2], f32)
            scal_t = wp.tile([128, 4], f32)

            # scratch + dummy gelu first: the gelu ACT_TABLE_LOAD runs
            # concurrently with the input DMAs.  The dummy reads and writes
            # disjoint cols of its own scratch (the write allocates the
            # tile) so the wu_sb memset / warm-ups need not wait for it.
            wu_sb = wp.tile([128, 128], f32, name="wu_sb")
            scr_t = wp.tile([1, 4], f32, name="scr_t")
            # tail tiles: engine APs must start at a 32-aligned partition,
            # so the absorb/atten pre-acts live in one [128, L] tile with
            # four aligned 16-row groups: abs(h) at rows 64h, att(h) at
            # rows 64h+32 (h = sample half); the gaps stay unused.
            aw_all = p2sb.tile([128, L], f32, name="aw_all")
            texpx = p2sb.tile([128, L], f32, name="texpx")
            outT = p2sb.tile([64, BL], f32, name="outT")
            outTT = p2sb.tile([64, BL], f32, name="outTT")

            # startup DMAs: first-needed tensors first, spread across the
            # three DGE queues (sync / scalar-hwdge / gpsimd-swdge)
            x2_pre = {}

            def fetch_x2(g):
                xt = xp.tile([128, 2 * L], bf16, name=f"x2_{g}", tag="x2")
                nc.sync.dma_start(xt[:], d["xlt"][:, g * 1024:(g + 1) * 1024])
                x2_pre[g] = xt

            fetch_x2(0)
            nc.scalar.dma_start(w1s_t[:], d["w1s"][:])
            nc.scalar.dma_start(b1_t[:], d["b1"][:])
            fetch_x2(1)
            fetch_x2(2)
            nc.scalar.activation(scr_t[0:1, 0:1], scr_t[0:1, 1:2], AF.Gelu)
            nc.gpsimd.memset(wu_sb[:], 0.0)
            nc.gpsimd.memset(texpx[:, L - 1:L], 1.0)
            nc.gpsimd.memset(outT[:], 0.0)
            for nm, t in [("w2f8", w2f8_t), ("b2", b2_t), ("wawt", wawt_t),
                          ("xgt", xgt_t), ("wd1g", wd1g_t), ("wd1p", wd1p_t),
                          ("wd2", wd2_t), ("bd1", bd1_t), ("scal", scal_t)]:
                nc.gpsimd.dma_start(t[:], d[nm][:])

            pooled_t = pp.tile([128, 4 * BL], f32)   # [h_part, mc*32 + s]
            h2all = pp.tile([128, NBLK * 4 * 2048], bf16)  # [feat, (g*4+mc)*2048 + j*512 + l]

            # all of PSUM as one tile; pairs P0=[0:2048], P1=[2048:4096]
            ps_all = psp.tile([128, 4096], f32)

            # PE warm-up: back-to-back matmuls on scratch data so the HAM
            # clock gate reaches K=8/8 before (and until) the real work.
            for i in range(8):
                nc.tensor.matmul(ps_all[:, 3968:4096], wu_sb[:], wu_sb[:],
                                 start=True, stop=True)

            h1tiles = {}
            unit = 0  # ACT-unit counter; parity picks the PSUM pair

            def filler(n):
                # keep the PE activity monitor (HAM) above its clock-gate
                # threshold; writes land in bank 7's tail, which the next
                # pair-1 unit overwrites with start=True (ordering-only).
                for i in range(n):
                    nc.tensor.matmul(ps_all[:, 3968:4096], wu_sb[:],
                                     wu_sb[:], start=True, stop=True)

            def l1_unit(g, mc):
                """One layer-1 ACT unit: 4 row-tiled bf16 MM pairs + gelu->fp8."""
                nonlocal unit
                x2t = x2_pre[g]
                h1t = h1tiles[g]
                pbase = (unit % 2) * 2048
                for h in range(2):
                    for sl in range(2):
                        nc.tensor.matmul(
                            ps_all[:, pbase + (2 * h + sl) * 512:
                                   pbase + (2 * h + sl) * 512 + 512],
                            w1s_t[64 * sl:64 * sl + 64, mc * 128:(mc + 1) * 128],
                            x2t[64 * sl:64 * sl + 64, h * 512:(h + 1) * 512],
                            start=True, stop=True)
                nc.scalar.activation(
                    h1t[:, mc * 2048:(mc + 1) * 2048],
                    ps_all[:, pbase:pbase + 2048],
                    AF.Gelu, bias=b1_t[:, mc:mc + 1])
                unit += 1

            def emit_reduce(g, mc):
                # per-sample pooling: sum over L on the DVE
                h2base = (g * 4 + mc) * 2048
                nc.vector.reduce_sum(
                    pooled_t[:, mc * BL + g * 4:mc * BL + g * 4 + 4],
                    h2all[:, h2base:h2base + 2048].rearrange(
                        "p (j n) -> p j n", j=4),
                    axis=AX.X)

            def l2_unit(g, mc, reduce=True):
                """One layer-2 ACT unit: 8 fp8 DoubleRow MMs + gelu + pooling."""
                nonlocal unit
                h1t = h1tiles[g]
                pbase = (unit % 2) * 2048
                for kcp in range(2):
                    wk = w2f8_t[:, kcp * 1024:(kcp + 1) * 1024].rearrange(
                        "p (ko mcm) -> p ko mcm", ko=2)
                    hk = h1t[:, (2 * kcp) * 2048:(2 * kcp + 2) * 2048].rearrange(
                        "p (ko n) -> p ko n", ko=2)
                    for j in range(4):
                        nc.tensor.matmul(
                            ps_all[:, pbase + j * 512:pbase + (j + 1) * 512],
                            wk[:, :, mc * 128:(mc + 1) * 128],
                            hk[:, :, j * 512:(j + 1) * 512],
                            start=(kcp == 0), stop=(kcp == 1),
                            perf_mode=DR)
                h2base = (g * 4 + mc) * 2048
                nc.scalar.activation(
                    h2all[:, h2base:h2base + 2048],
                    ps_all[:, pbase:pbase + 2048],
                    AF.Gelu, bias=b2_t[:, mc:mc + 1])
                unit += 1
                if reduce:
                    emit_reduce(g, mc)

            def burst_mm(b, col):
                # absorb/atten pre-acts for block b: col-tiled burst into
                # psum cols [col, col+512), 4 samples in 32-col PE strips
                for c in range(4):
                    for j in range(4):
                        nc.tensor.matmul(
                            ps_all[32 * j:32 * j + 2, col:col + 512],
                            wawt_t[:, 2 * c:2 * c + 2],
                            h2all[:, (b * 4 + c) * 2048 + j * 512:
                                  (b * 4 + c) * 2048 + (j + 1) * 512],
                            start=(c == 0), stop=(c == 3),
                            tile_position=(0, 32 * j))

            def burst_drain(b, col, eng):
                aw_sb = awp.tile([128, L], f32, name=f"aw_sb_{b}", tag="aw_sb")
                if eng == "scalar":
                    nc.scalar.copy(aw_sb[:], ps_all[:, col:col + 512])
                else:
                    nc.vector.tensor_copy(aw_sb[:], ps_all[:, col:col + 512])
                h, r = b // 4, (b % 4) * 4
                nc.sync.dma_start(aw_all[64 * h + r:64 * h + r + 4, :],
                                  aw_sb[0:128:32, :])
                nc.sync.dma_start(aw_all[64 * h + 32 + r:64 * h + 32 + r + 4, :],
                                  aw_sb[1:128:32, :])

            # prologue: block 0's layer 1, with fillers to keep HAM at 8/8
            # through the l1-only (PE-light) stretch
            h1tiles[0] = h1p.tile([128, 4 * 2048], f8e4, name="h1t_0", tag="h1t")
            for mc in range(4):
                l1_unit(0, mc)
                filler(5)

            # main loop, software-pipelined: block g+1's layer-1 units
            # interleave with block g's layer-2 units so the PE never idles
            # long enough for HAM to re-throttle.  Block g-1's absorb/atten
            # burst rides in the pair-0 bank-3 window between l2(g,0) and
            # l1(g+1,1): it waits for l1(g+1,0)'s gelu to drain pair 0,
            # and its DVE drain completes before l1(g+1,1)'s bank-3 matmul
            # needs the bank back -- the scalar stream never notices.
            for g in range(NBLK):
                if g + 3 < NBLK:
                    fetch_x2(g + 3)
                if g + 1 < NBLK:
                    h1tiles[g + 1] = h1p.tile([128, 4 * 2048], f8e4,
                                              name=f"h1t_{g+1}", tag="h1t")
                for mc in range(4):
                    if g + 1 < NBLK:
                        l1_unit(g + 1, mc)
                    l2_unit(g, mc, reduce=not (mc == 3 and g <= 5))
                    if mc == 0 and 1 <= g <= 6:
                        burst_mm(g - 1, 1536)
                        burst_drain(g - 1, 1536, "dve")
                        emit_reduce(g - 1, 3)
                    if g < 2:
                        filler(2)

            # ---- phase 2 ----
            # Only blocks 6 and 7 still need their absorb/atten bursts
            # (0-5 ran mid-loop); they use pair-0 banks 0/1, free once the
            # second-to-last gelu drains, with scalar-engine drains.
            burst_mm(6, 0)
            burst_mm(7, 512)
            burst_drain(6, 0, "scalar")
            burst_drain(7, 512, "scalar")

            # dense head: the Xg part and blocks 0-6 of the pooled part
            # accumulate as soon as the PE is free (sample cols 0:28); only
            # block 7's pooled columns (28:32) wait for the last reduce.
            d1in = [ps_all[:, 1024:1024 + BL], ps_all[:, 1536 + 64:1536 + 64 + BL]]
            for mc2 in range(2):
                for lo, hi in ((0, 28), (28, 32)):
                    ps = d1in[mc2][:, lo:hi]
                    nc.tensor.matmul(ps, wd1g_t[:, mc2 * 128:(mc2 + 1) * 128],
                                     xgt_t[:, lo:hi], start=True, stop=False)
                    for hc in range(4):
                        nc.tensor.matmul(
                            ps,
                            wd1p_t[:, hc * DH + mc2 * 128:hc * DH + (mc2 + 1) * 128],
                            pooled_t[:, hc * BL + lo:hc * BL + hi],
                            start=False, stop=(hc == 3))
            d1t = []
            gelu_insts = []
            for mc2 in range(2):
                t = p2sb.tile([128, BL], f32, name=f"d1t_{mc2}")
                gi = nc.scalar.activation(t[:], d1in[mc2], AF.Gelu,
                                          bias=bd1_t[:, mc2:mc2 + 1])
                gelu_insts.append(gi)
                d1t.append(t)

            # single table switch to the ln/exp set, after the last gelu
            from concourse.hw_specs import get_activation_tables
            tabs = get_activation_tables(nc.m.arch)
            set_id = next(i for i, fns in enumerate(tabs.values())
                          if AF.Exp in fns and AF.Ln in fns)
            li = nc.scalar.add_instruction(mybir.InstLoadActFuncSet(
                name=nc.scalar.bass.get_next_instruction_name(),
                act_func_set_id=set_id, ins=[], outs=[]))
            for gi in gelu_insts:
                add_dep_helper(li.ins, gi.ins, sync=True,
                               reason="ACT table set order: gelu before ln/exp")

            # per-half wd2 contraction so each half's d starts at partition 0
            dps_h = [ps_all[0:16, 2048:2049], ps_all[0:16, 2560:2561]]
            for h in range(2):
                nc.tensor.matmul(dps_h[h], d1t[0][:, 16 * h:16 * h + 16],
                                 wd2_t[:, 0:1], start=True, stop=False)
                nc.tensor.matmul(dps_h[h], d1t[1][:, 16 * h:16 * h + 16],
                                 wd2_t[:, 1:2], start=False, stop=True)

            # tail, one pass over the [128, L] aw tile (all 32 samples):
            # softplus via ln(1+exp), T via exp(-cumsum) of the reversed
            # atten scan; garbage rows compute for free.
            e_aw = p2sb.tile([128, L], f32, name="e_aw")
            sp_t = p2sb.tile([128, L], f32, name="sp_t")
            incl = p2sb.tile([128, L], f32, name="incl")
            contrib = p2sb.tile([128, L], f32, name="contrib")
            cap = p2sb.tile([128, 1], f32, name="cap")
            ei = nc.scalar.activation(e_aw[:], aw_all[:], AF.Exp,
                                      bias=scal_t[:, 0:1])
            add_dep_helper(ei.ins, li.ins, sync=True,
                           reason="ACT table set order: ln set before exp")
            nc.scalar.activation(sp_t[:], e_aw[:], AF.Ln, bias=1.0)
            # incl[l] = sum_{l'>=l} softplus(atten): add-scan over reversed
            # L, shifting the atten rows down 32 partitions so T lands on
            # the same partitions as the absorb rows (the STT below needs
            # both SBUF inputs at one base partition; >32-partition
            # patterns must start at partition 0, hence per-half scans).
            for h in range(2):
                spt_rev = sp_t[64 * h + 32:64 * h + 48, L - 1::-1]
                incl_rev = incl[64 * h:64 * h + 16, L - 1::-1]
                nc.vector.tensor_tensor_scan(incl_rev, spt_rev, spt_rev, 0.0,
                                             ALU.add, ALU.bypass)
                # texpx[l] = T[l] = exp(-incl[l+1]); col L-1 pre-set to 1.0
                nc.scalar.activation(texpx[64 * h:64 * h + 16, 0:L - 1],
                                     incl[64 * h:64 * h + 16, 1:L],
                                     AF.Exp, scale=-1.0)
            for h in range(2):
                nc.vector.scalar_tensor_tensor(
                    contrib[64 * h:64 * h + 16, :],
                    sp_t[64 * h:64 * h + 16, :], 1.0,
                    texpx[64 * h:64 * h + 16, :], ALU.mult, ALU.mult,
                    accum_out=cap[64 * h:64 * h + 16, 0:1])
            for h in range(2):
                # out = (cap + bd2) + d, into rows 32h:32h+16 of outT col 0
                nc.vector.scalar_tensor_tensor(
                    outT[32 * h:32 * h + 16, 0:1],
                    cap[64 * h:64 * h + 16, 0:1],
                    scal_t[64 * h:64 * h + 16, 2:3], dps_h[h],
                    ALU.add, ALU.add)
            # transpose so the output DMA is two contiguous descriptors
            nc.vector.transpose(outTT[:], outT[:])
            nc.sync.dma_start(out_d[:], outTT[0:64:32, 0:16])

    nc.compile()
    return nc


_CACHE = {}


def _prep_inputs(inputs):
    f = lambda a: np.ascontiguousarray(np.asarray(a, dtype=np.float32))
    Xg, Xl = f(inputs["Xg"]), f(inputs["Xl"])
    W1, b1 = f(inputs["W1"]), f(inputs["b1"])
    W2, b2 = f(inputs["W2"]), f(inputs["b2"])
    wa, ba = f(inputs["wa"]), f(inputs["ba"])
    wt, bt = f(inputs["wt"]), f(inputs["bt"])
    Wd1, bd1 = f(inputs["Wd1"]), f(inputs["bd1"])
    Wd2, bd2 = f(inputs["Wd2"]), f(inputs["bd2"])

    shared = {
        "w1s": np.ascontiguousarray(np.concatenate([W1, W1], axis=0)).astype(ml_dtypes.bfloat16),
        # [k, kc', ko, mc, m]: W2 row = kc'*256 + ko*128 + k, col = mc*128 + m
        "w2f8": np.ascontiguousarray(
            W2.reshape(2, 2, 128, 4, 128).transpose(2, 0, 1, 3, 4)
            .reshape(128, 2048)).astype(ml_dtypes.float8_e4m3),
        "wawt": np.ascontiguousarray(
            np.concatenate([wa, wt], axis=1).reshape(4, 128, 2)
            .transpose(1, 0, 2).reshape(128, 8)).astype(ml_dtypes.bfloat16),
        "wd1g": np.ascontiguousarray(Wd1[:G]),
        "wd1p": np.ascontiguousarray(
            (Wd1[G:] / np.float32(L)).reshape(4, 128, DH)
            .transpose(1, 0, 2).reshape(128, 4 * DH)),
        "wd2": np.ascontiguousarray(Wd2.reshape(2, 128).T),
        "b1": np.ascontiguousarray(b1.reshape(4, 128).T),
        "b2": np.ascontiguousarray(b2.reshape(4, 128).T),
        "bd1": np.ascontiguousarray(bd1.reshape(2, 128).T),
    }
    scal = np.zeros((128, 4), np.float32)
    for h in range(2):
        scal[64 * h:64 * h + 16, 0] = ba.reshape(-1)[0]
        scal[64 * h + 32:64 * h + 48, 0] = bt.reshape(-1)[0]
    scal[:, 2] = bd2.reshape(-1)[0]
    shared["scal"] = scal

    in_maps = []
    for c in range(NCORES):
        s = slice(c * BL, (c + 1) * BL)
        m = dict(shared)
        # [sl*64+f, g*1024 + h*512 + l]: 2KB-contiguous per partition row
        # per block so each block's fetch is 128 descriptors, not 256
        m["xlt"] = np.ascontiguousarray(
            Xl[s].reshape(NBLK, 2, 2, L, FD).transpose(2, 4, 0, 1, 3)
            .reshape(128, NBLK * 1024)).astype(ml_dtypes.bfloat16)
        m["xgt"] = np.ascontiguousarray(Xg[s].T)
        in_maps.append(m)
    return in_maps


def _run(inputs, trace=False, tmpdir=None):
    if "nc" not in _CACHE:
        _CACHE["nc"] = _build()
    nc = _CACHE["nc"]
    in_maps = _prep_inputs(inputs)
    res = run_bass_kernel_spmd(nc, in_maps, list(range(NCORES)),
                               trace=trace, tmpdir=tmpdir)
    out = np.concatenate([res.results[c]["out"].reshape(BL, 1)
                          for c in range(NCORES)], axis=0)
    return out.astype(np.float32), res


def kernel(**inputs) -> np.ndarray:
    out, _ = _run(inputs)
    return out


# revision 27
# speedup vs baseline: 1.1032x; 1.1032x over previous
"""Trainium2 Bass kernel for nn_CumulativeShadeRegressor.

Model (per sample): per-leaf MLP encoder [L, FD] -> [L, H2] (two gelu
layers), softplus absorb/atten heads, a top-to-bottom exponential
transmittance scan over L, mean-pooling over L, and a small dense head on
[Xg | pooled].

Strategy: data-parallel over B across 8 NeuronCores (32 samples/core).
The ACT (scalar) engine is the bottleneck: 64 gelu ACTIVATEs of 2048 cols
each (~2us apiece) form a ~126us stream that everything else must hide
under.  The kernel therefore:
  * layer 1 (K=64) runs as row-tiled bf16 matmul pairs (2 concurrent MMs
    in disjoint 64-row PE strips);
  * layer 2 runs in fp8e4 DoubleRow mode (2 MACs/cell, contraction 256
    per pass) with h1 quantized to fp8 by the gelu ACT itself;
  * gelu ACT ops are batched to N=2048 (4 samples per instruction, PSUM
    pair ring of 2x4 banks) to amortize the ~300-cycle ACT init;
  * startup DMAs are spread across all three DGE queues (sync, scalar,
    gpsimd) so the first x2 block + w1s land ~3us earlier;
  * PE warm-up/filler matmuls keep the HAM clock gate at K=8/8 through
    the l1-only prologue (a K=4/8 dip used to stall the ACT stream 4.6us);
  * phase 2 gives each absorb/atten burst its own PSUM bank (no ping-pong
    chains), drains 4 bursts on the scalar engine and 4 on the DVE, and
    runs the softplus/scan/transmittance tail in two pipelined halves
    (samples 0-15 and 16-31) so scalar, DVE, and DMA latency overlap;
  * the final [32,1] result is transposed to one SBUF row so the output
    DMA is a single descriptor (the 32-descriptor version left its
    completion semaphore trickling for ~5us after the data landed).
"""
import sys

sys.path.insert(0, "/opt/trn_rl_repo")

import numpy as np
import ml_dtypes

import concourse.bacc as bacc
import concourse.mybir as mybir
import concourse.tile as tile
from concourse.bass_utils import run_bass_kernel_spmd
from concourse.tile import add_dep_helper

B, L, FD, G = 256, 512, 64, 32
H1, H2, DH = 512, 512, 256
NCORES = 8
BL = B // NCORES          # 32 samples per core
NBLK = BL // 4            # 8 blocks of 4 samples

f32 = mybir.dt.float32
bf16 = mybir.dt.bfloat16
f8e4 = mybir.dt.float8e4
AF = mybir.ActivationFunctionType
ALU = mybir.AluOpType
AX = mybir.AxisListType
DR = mybir.MatmulPerfMode.DoubleRow


def _build():
    nc = bacc.Bacc("TRN2", target_bir_lowering=False, debug=False,
                   num_devices=NCORES)

    d = {}
    d["xlt"] = nc.dram_tensor("xlt", [128, NBLK * 1024], bf16, kind="ExternalInput").ap()
    d["xgt"] = nc.dram_tensor("xgt", [G, BL], f32, kind="ExternalInput").ap()
    d["w1s"] = nc.dram_tensor("w1s", [128, H1], bf16, kind="ExternalInput").ap()
    d["w2f8"] = nc.dram_tensor("w2f8", [128, 2048], f8e4, kind="ExternalInput").ap()
    d["wawt"] = nc.dram_tensor("wawt", [128, 8], bf16, kind="ExternalInput").ap()
    d["wd1g"] = nc.dram_tensor("wd1g", [G, DH], f32, kind="ExternalInput").ap()
    d["wd1p"] = nc.dram_tensor("wd1p", [128, 4 * DH], f32, kind="ExternalInput").ap()
    d["wd2"] = nc.dram_tensor("wd2", [128, 2], f32, kind="ExternalInput").ap()
    d["b1"] = nc.dram_tensor("b1", [128, 4], f32, kind="ExternalInput").ap()
    d["b2"] = nc.dram_tensor("b2", [128, 4], f32, kind="ExternalInput").ap()
    d["bd1"] = nc.dram_tensor("bd1", [128, 2], f32, kind="ExternalInput").ap()
    d["scal"] = nc.dram_tensor("scal", [128, 4], f32, kind="ExternalInput").ap()
    out_d = nc.dram_tensor("out", [2, BL // 2], f32, kind="ExternalOutput").ap()

    with tile.TileContext(nc) as tc:
        with (
            tc.tile_pool(name="wp", bufs=1) as wp,
            tc.tile_pool(name="pp", bufs=1) as pp,
            tc.tile_pool(name="xp", bufs=4) as xp,
            tc.tile_pool(name="h1p", bufs=2) as h1p,
            tc.tile_pool(name="p2sb", bufs=1) as p2sb,
            tc.tile_pool(name="awp", bufs=4) as awp,
            tc.tile_pool(name="psp", bufs=1, space="PSUM") as psp,
        ):
            w1s_t = wp.tile([128, H1], bf16)
            w2f8_t = wp.tile([128, 2048], f8e4)
            wawt_t = wp.tile([128, 8], bf16)
            xgt_t = wp.tile([G, BL], f32)
            wd1g_t = wp.tile([G, DH], f32)
            wd1p_t = wp.tile([128, 4 * DH], f32)
            wd2_t = wp.tile([128, 2], f32)
            b1_t = wp.tile([128, 4], f32)
            b2_t = wp.tile([128, 4], f32)
            bd1_t = wp.tile([128, 2], f32)
            scal_t = wp.tile([128, 4], f32)

            # scratch + dummy gelu first: the gelu ACT_TABLE_LOAD runs
            # concurrently with the input DMAs.  The dummy reads and writes
            # disjoint cols of its own scratch (the write allocates the
            # tile) so the wu_sb memset / warm-ups need not wait for it.
            wu_sb = wp.tile([128, 128], f32, name="wu_sb")
            scr_t = wp.tile([1, 4], f32, name="scr_t")
            # tail tiles: engine APs must start at a 32-aligned partition,
            # so the absorb/atten pre-acts live in one [128, L] tile with
            # four aligned 16-row groups: abs(h) at rows 64h, att(h) at
            # rows 64h+32 (h = sample half); the gaps stay unused.
            aw_all = p2sb.tile([128, L], f32, name="aw_all")
            texpx = p2sb.tile([128, L], f32, name="texpx")
            outT = p2sb.tile([64, BL], f32, name="outT")
            outTT = p2sb.tile([64, BL], f32, name="outTT")

            # startup DMAs: first-needed tensors first, spread across the
            # three DGE queues (sync / scalar-hwdge / gpsimd-swdge)
            x2_pre = {}

            def fetch_x2(g):
                xt = xp.tile([128, 2 * L], bf16, name=f"x2_{g}", tag="x2")
                nc.sync.dma_start(xt[:], d["xlt"][:, g * 1024:(g + 1) * 1024])
                x2_pre[g] = xt

            fetch_x2(0)
            nc.scalar.dma_start(w1s_t[:], d["w1s"][:])
            nc.scalar.dma_start(b1_t[:], d["b1"][:])
            fetch_x2(1)
            fetch_x2(2)
            nc.scalar.activation(scr_t[0:1, 0:1], scr_t[0:1, 1:2], AF.Gelu)
            nc.gpsimd.memset(wu_sb[:], 0.0)
            nc.gpsimd.memset(texpx[:, L - 1:L], 1.0)
            nc.gpsimd.memset(outT[:], 0.0)
            for nm, t in [("w2f8", w2f8_t), ("b2", b2_t), ("wawt", wawt_t),
                          ("xgt", xgt_t), ("wd1g", wd1g_t), ("wd1p", wd1p_t),
                          ("wd2", wd2_t), ("bd1", bd1_t), ("scal", scal_t)]:
                nc.gpsimd.dma_start(t[:], d[nm][:])

            pooled_t = pp.tile([128, 4 * BL], f32)   # [h_part, mc*32 + s]
            h2all = pp.tile([128, NBLK * 4 * 2048], bf16)  # [feat, (g*4+mc)*2048 + j*512 + l]

            # all of PSUM as one tile; pairs P0=[0:2048], P1=[2048:4096]
            ps_all = psp.tile([128, 4096], f32)

            # PE warm-up: back-to-back matmuls on scratch data so the HAM
            # clock gate reaches K=8/8 before (and until) the real work.
            for i in range(8):
                nc.tensor.matmul(ps_all[:, 3968:4096], wu_sb[:], wu_sb[:],
                                 start=True, stop=True)

            h1tiles = {}
            unit = 0  # ACT-unit counter; parity picks the PSUM pair

            def filler(n):
                # keep the PE activity monitor (HAM) above its clock-gate
                # threshold; writes land in bank 7's tail, which the next
                # pair-1 unit overwrites with start=True (ordering-only).
                for i in range(n):
                    nc.tensor.matmul(ps_all[:, 3968:4096], wu_sb[:],
                                     wu_sb[:], start=True, stop=True)

            def l1_unit(g, mc):
                """One layer-1 ACT unit: 4 row-tiled bf16 MM pairs + gelu->fp8."""
                nonlocal unit
                x2t = x2_pre[g]
                h1t = h1tiles[g]
                pbase = (unit % 2) * 2048
                for h in range(2):
                    for sl in range(2):
                        nc.tensor.matmul(
                            ps_all[:, pbase + (2 * h + sl) * 512:
                                   pbase + (2 * h + sl) * 512 + 512],
                            w1s_t[64 * sl:64 * sl + 64, mc * 128:(mc + 1) * 128],
                            x2t[64 * sl:64 * sl + 64, h * 512:(h + 1) * 512],
                            start=True, stop=True)
                nc.scalar.activation(
                    h1t[:, mc * 2048:(mc + 1) * 2048],
                    ps_all[:, pbase:pbase + 2048],
                    AF.Gelu, bias=b1_t[:, mc:mc + 1])
                unit += 1

            def emit_reduce(g, mc):
                # per-sample pooling: sum over L on the DVE
                h2base = (g * 4 + mc) * 2048
                nc.vector.reduce_sum(
                    pooled_t[:, mc * BL + g * 4:mc * BL + g * 4 + 4],
                    h2all[:, h2base:h2base + 2048].rearrange(
                        "p (j n) -> p j n", j=4),
                    axis=AX.X)

            def l2_unit(g, mc, reduce=True):
                """One layer-2 ACT unit: 8 fp8 DoubleRow MMs + gelu + pooling."""
                nonlocal unit
                h1t = h1tiles[g]
                pbase = (unit % 2) * 2048
                for kcp in range(2):
                    wk = w2f8_t[:, kcp * 1024:(kcp + 1) * 1024].rearrange(
                        "p (ko mcm) -> p ko mcm", ko=2)
                    hk = h1t[:, (2 * kcp) * 2048:(2 * kcp + 2) * 2048].rearrange(
                        "p (ko n) -> p ko n", ko=2)
                    for j in range(4):
                        nc.tensor.matmul(
                            ps_all[:, pbase + j * 512:pbase + (j + 1) * 512],
                            wk[:, :, mc * 128:(mc + 1) * 128],
                            hk[:, :, j * 512:(j + 1) * 512],
                            start=(kcp == 0), stop=(kcp == 1),
                            perf_mode=DR)
                h2base = (g * 4 + mc) * 2048
                nc.scalar.activation(
                    h2all[:, h2base:h2base + 2048],
                    ps_all[:, pbase:pbase + 2048],
                    AF.Gelu, bias=b2_t[:, mc:mc + 1])
                unit += 1
                if reduce:
                    emit_reduce(g, mc)

            def burst_mm(b, col):
                # absorb/atten pre-acts for block b: col-tiled burst into
                # psum cols [col, col+512), 4 samples in 32-col PE strips
                for c in range(4):
                    for j in range(4):
                        nc.tensor.matmul(
                            ps_all[32 * j:32 * j + 2, col:col + 512],
                            wawt_t[:, 2 * c:2 * c + 2],
                            h2all[:, (b * 4 + c) * 2048 + j * 512:
                                  (b * 4 + c) * 2048 + (j + 1) * 512],
                            start=(c == 0), stop=(c == 3),
                            tile_position=(0, 32 * j))

            def burst_drain(b, col, eng):
                aw_sb = awp.tile([128, L], f32, name=f"aw_sb_{b}", tag="aw_sb")
                if eng == "scalar":
                    nc.scalar.copy(aw_sb[:], ps_all[:, col:col + 512])
                else:
                    nc.vector.tensor_copy(aw_sb[:], ps_all[:, col:col + 512])
                h, r = b // 4, (b % 4) * 4
                nc.sync.dma_start(aw_all[64 * h + r:64 * h + r + 4, :],
                                  aw_sb[0:128:32, :])
                nc.sync.dma_start(aw_all[64 * h + 32 + r:64 * h + 32 + r + 4, :],
                                  aw_sb[1:128:32, :])

            # prologue: block 0's layer 1, with fillers (before the first
            # pair-1 reader only -- later filler slots would FIFO-block the
            # 0.1us-slack l2 matmul chains) to keep HAM from throttling.
            h1tiles[0] = h1p.tile([128, 4 * 2048], f8e4, name="h1t_0", tag="h1t")
            l1_unit(0, 0)
            filler(12)
            for mc in range(1, 4):
                l1_unit(0, mc)

            # main loop, software-pipelined: block g+1's layer-1 units
            # interleave with block g's layer-2 units.  The loop is kept
            # pristine: every l2 matmul chain (1.9us) must start the
            # instant the previous l2 gelu drains pair 1 (0.1us slack), so
            # no extra PE work may ride inside the loop.
            for g in range(NBLK):
                if g + 3 < NBLK:
                    fetch_x2(g + 3)
                if g + 1 < NBLK:
                    h1tiles[g + 1] = h1p.tile([128, 4 * 2048], f8e4,
                                              name=f"h1t_{g+1}", tag="h1t")
                for mc in range(4):
                    if g + 1 < NBLK:
                        l1_unit(g + 1, mc)
                    l2_unit(g, mc)

            # ---- phase 2 ----
            # Burst schedule: pair-0 banks free 2us before the stream ends
            # (second-to-last gelu), pair-1 banks at the end.  b0/b1 pre-run
            # on banks 1/2; b7 (whose data is only ready at the very end)
            # gets bank 0 and the first post-stream PE slot so half 1's aw
            # rows start draining immediately; the rest follow.  The dense
            # head rides in bank 7 between b3 and b4 so its gelus + the
            # table switch complete while the late bursts still drain.
            for b in range(4):     # banks 0-3 free once gelu62 drains pair 0
                burst_mm(b, 512 * b)
            burst_mm(7, 3584)      # bank 7; first pair-1 slot after gelu63

            d1in = [ps_all[:, 0:BL], ps_all[:, 512:512 + BL]]
            for lo, hi in ((0, 28), (28, 32)):
                # Xg part + pooled blocks 0-6 accumulate as soon as banks
                # 0/1 drain (sample cols 0:28); only block 7's pooled
                # columns (28:32) wait for the last pooling reduce.
                for mc2 in range(2):
                    ps = d1in[mc2][:, lo:hi]
                    nc.tensor.matmul(ps, wd1g_t[:, mc2 * 128:(mc2 + 1) * 128],
                                     xgt_t[:, lo:hi], start=True, stop=False)
                    for hc in range(4):
                        nc.tensor.matmul(
                            ps,
                            wd1p_t[:, hc * DH + mc2 * 128:hc * DH + (mc2 + 1) * 128],
                            pooled_t[:, hc * BL + lo:hc * BL + hi],
                            start=False, stop=(hc == 3))

            burst_mm(4, 2048)      # banks 4-6
            burst_mm(5, 2560)
            burst_mm(6, 3072)

            # scalar drains half 0 + b7 while the DVE (busy with the last
            # pooling reduce first) takes the rest of half 1
            for b in range(4):
                burst_drain(b, 512 * b, "scalar")
            burst_drain(7, 3584, "scalar")
            burst_drain(4, 2048, "dve")
            burst_drain(5, 2560, "dve")
            burst_drain(6, 3072, "dve")

            d1t = []
            gelu_insts = []
            for mc2 in range(2):
                t = p2sb.tile([128, BL], f32, name=f"d1t_{mc2}")
                gi = nc.scalar.activation(t[:], d1in[mc2], AF.Gelu,
                                          bias=bd1_t[:, mc2:mc2 + 1])
                gelu_insts.append(gi)
                d1t.append(t)

            # single table switch to the ln/exp set, after the last gelu
            from concourse.hw_specs import get_activation_tables
            tabs = get_activation_tables(nc.m.arch)
            set_id = next(i for i, fns in enumerate(tabs.values())
                          if AF.Exp in fns and AF.Ln in fns)
            li = nc.scalar.add_instruction(mybir.InstLoadActFuncSet(
                name=nc.scalar.bass.get_next_instruction_name(),
                act_func_set_id=set_id, ins=[], outs=[]))
            for gi in gelu_insts:
                add_dep_helper(li.ins, gi.ins, sync=True,
                               reason="ACT table set order: gelu before ln/exp")

            # per-half wd2 contraction so each half's d starts at partition 0
            dps_h = [ps_all[0:16, 1024:1025], ps_all[0:16, 1088:1089]]
            for h in range(2):
                nc.tensor.matmul(dps_h[h], d1t[0][:, 16 * h:16 * h + 16],
                                 wd2_t[:, 0:1], start=True, stop=False)
                nc.tensor.matmul(dps_h[h], d1t[1][:, 16 * h:16 * h + 16],
                                 wd2_t[:, 1:2], start=False, stop=True)

            # tail, one pass over the [128, L] aw tile (all 32 samples):
            # softplus via ln(1+exp), T via exp(-cumsum) of the reversed
            # atten scan; garbage rows compute for free.
            e_aw = p2sb.tile([128, L], f32, name="e_aw")
            sp_t = p2sb.tile([128, L], f32, name="sp_t")
            incl = p2sb.tile([128, L], f32, name="incl")
            contrib = p2sb.tile([128, L], f32, name="contrib")
            cap = p2sb.tile([128, 1], f32, name="cap")
            # per-half softplus so half 0 (drained first) starts while the
            # late bursts are still landing
            ei = nc.scalar.activation(e_aw[0:64, :], aw_all[0:64, :], AF.Exp,
                                      bias=scal_t[0:64, 0:1])
            add_dep_helper(ei.ins, li.ins, sync=True,
                           reason="ACT table set order: ln set before exp")
            nc.scalar.activation(sp_t[0:64, :], e_aw[0:64, :], AF.Ln, bias=1.0)
            nc.scalar.activation(e_aw[64:128, :], aw_all[64:128, :], AF.Exp,
                                 bias=scal_t[64:128, 0:1])
            nc.scalar.activation(sp_t[64:128, :], e_aw[64:128, :], AF.Ln,
                                 bias=1.0)
            # incl[l] = sum_{l'>=l} softplus(atten): add-scan over reversed
            # L, shifting the atten rows down 32 partitions so T lands on
            # the same partitions as the absorb rows (the STT below needs
            # both SBUF inputs at one base partition; >32-partition
            # patterns must start at partition 0, hence per-half scans).
            for h in range(2):
                spt_rev = sp_t[64 * h + 32:64 * h + 48, L - 1::-1]
                incl_rev = incl[64 * h:64 * h + 16, L - 1::-1]
                nc.vector.tensor_tensor_scan(incl_rev, spt_rev, spt_rev, 0.0,
                                             ALU.add, ALU.bypass)
                # texpx[l] = T[l] = exp(-incl[l+1]); col L-1 pre-set to 1.0
                nc.scalar.activation(texpx[64 * h:64 * h + 16, 0:L - 1],
                                     incl[64 * h:64 * h + 16, 1:L],
                                     AF.Exp, scale=-1.0)
            for h in range(2):
                nc.vector.scalar_tensor_tensor(
                    contrib[64 * h:64 * h + 16, :],
                    sp_t[64 * h:64 * h + 16, :], 1.0,
                    texpx[64 * h:64 * h + 16, :], ALU.mult, ALU.mult,
                    accum_out=cap[64 * h:64 * h + 16, 0:1])
            for h in range(2):
                # out = (cap + bd2) + d, into rows 32h:32h+16 of outT col 0
                nc.vector.scalar_tensor_tensor(
                    outT[32 * h:32 * h + 16, 0:1],
                    cap[64 * h:64 * h + 16, 0:1],
                    scal_t[64 * h:64 * h + 16, 2:3], dps_h[h],
                    ALU.add, ALU.add)
            # transpose so the output DMA is two contiguous descriptors
            nc.vector.transpose(outTT[:], outT[:])
            nc.sync.dma_start(out_d[:], outTT[0:64:32, 0:16])

    nc.compile()
    return nc


_CACHE = {}


def _prep_inputs(inputs):
    f = lambda a: np.ascontiguousarray(np.asarray(a, dtype=np.float32))
    Xg, Xl = f(inputs["Xg"]), f(inputs["Xl"])
    W1, b1 = f(inputs["W1"]), f(inputs["b1"])
    W2, b2 = f(inputs["W2"]), f(inputs["b2"])
    wa, ba = f(inputs["wa"]), f(inputs["ba"])
    wt, bt = f(inputs["wt"]), f(inputs["bt"])
    Wd1, bd1 = f(inputs["Wd1"]), f(inputs["bd1"])
    Wd2, bd2 = f(inputs["Wd2"]), f(inputs["bd2"])

    shared = {
        "w1s": np.ascontiguousarray(np.concatenate([W1, W1], axis=0)).astype(ml_dtypes.bfloat16),
        # [k, kc', ko, mc, m]: W2 row = kc'*256 + ko*128 + k, col = mc*128 + m
        "w2f8": np.ascontiguousarray(
            W2.reshape(2, 2, 128, 4, 128).transpose(2, 0, 1, 3, 4)
            .reshape(128, 2048)).astype(ml_dtypes.float8_e4m3),
        "wawt": np.ascontiguousarray(
            np.concatenate([wa, wt], axis=1).reshape(4, 128, 2)
            .transpose(1, 0, 2).reshape(128, 8)).astype(ml_dtypes.bfloat16),
        "wd1g": np.ascontiguousarray(Wd1[:G]),
        "wd1p": np.ascontiguousarray(
            (Wd1[G:] / np.float32(L)).reshape(4, 128, DH)
            .transpose(1, 0, 2).reshape(128, 4 * DH)),
        "wd2": np.ascontiguousarray(Wd2.reshape(2, 128).T),
        "b1": np.ascontiguousarray(b1.reshape(4, 128).T),
        "b2": np.ascontiguousarray(b2.reshape(4, 128).T),
        "bd1": np.ascontiguousarray(bd1.reshape(2, 128).T),
    }
    scal = np.zeros((128, 4), np.float32)
    for h in range(2):
        scal[64 * h:64 * h + 16, 0] = ba.reshape(-1)[0]
        scal[64 * h + 32:64 * h + 48, 0] = bt.reshape(-1)[0]
    scal[:, 2] = bd2.reshape(-1)[0]
    shared["scal"] = scal

    in_maps = []
    for c in range(NCORES):
        s = slice(c * BL, (c + 1) * BL)
        m = dict(shared)
        # [sl*64+f, g*1024 + h*512 + l]: 2KB-contiguous per partition row
        # per block so each block's fetch is 128 descriptors, not 256
        m["xlt"] = np.ascontiguousarray(
            Xl[s].reshape(NBLK, 2, 2, L, FD).transpose(2, 4, 0, 1, 3)
            .reshape(128, NBLK * 1024)).astype(ml_dtypes.bfloat16)
        m["xgt"] = np.ascontiguousarray(Xg[s].T)
        in_maps.append(m)
    return in_maps


def _run(inputs, trace=False, tmpdir=None):
    if "nc" not in _CACHE:
        _CACHE["nc"] = _build()
    nc = _CACHE["nc"]
    in_maps = _prep_inputs(inputs)
    res = run_bass_kernel_spmd(nc, in_maps, list(range(NCORES)),
                               trace=trace, tmpdir=tmpdir)
    out = np.concatenate([res.results[c]["out"].reshape(BL, 1)
                          for c in range(NCORES)], axis=0)
    return out.astype(np.float32), res


def kernel(**inputs) -> np.ndarray:
    out, _ = _run(inputs)
    return out


# revision 28
# speedup vs baseline: 1.1358x; 1.0295x over previous
"""Trainium2 Bass kernel for nn_CumulativeShadeRegressor.

Model (per sample): per-leaf MLP encoder [L, FD] -> [L, H2] (two gelu
layers), softplus absorb/atten heads, a top-to-bottom exponential
transmittance scan over L, mean-pooling over L, and a small dense head on
[Xg | pooled].

Strategy: data-parallel over B across 8 NeuronCores (32 samples/core).
The ACT (scalar) engine is the bottleneck: 64 gelu ACTIVATEs of 2048 cols
each (~2us apiece) form a ~126us stream that everything else must hide
under.  The kernel therefore:
  * layer 1 (K=64) runs as row-tiled bf16 matmul pairs (2 concurrent MMs
    in disjoint 64-row PE strips);
  * layer 2 runs in fp8e4 DoubleRow mode (2 MACs/cell, contraction 256
    per pass) with h1 quantized to fp8 by the gelu ACT itself;
  * gelu ACT ops are batched to N=2048 (4 samples per instruction, PSUM
    pair ring of 2x4 banks) to amortize the ~300-cycle ACT init;
  * startup DMAs are spread across all three DGE queues (sync, scalar,
    gpsimd) so the first x2 block + w1s land ~3us earlier;
  * PE warm-up/filler matmuls keep the HAM clock gate at K=8/8 through
    the l1-only prologue (a K=4/8 dip used to stall the ACT stream 4.6us);
  * phase 2 gives each absorb/atten burst its own PSUM bank (no ping-pong
    chains), drains 4 bursts on the scalar engine and 4 on the DVE, and
    runs the softplus/scan/transmittance tail in two pipelined halves
    (samples 0-15 and 16-31) so scalar, DVE, and DMA latency overlap;
  * the final [32,1] result is transposed to one SBUF row so the output
    DMA is a single descriptor (the 32-descriptor version left its
    completion semaphore trickling for ~5us after the data landed).
"""
import sys

sys.path.insert(0, "/opt/trn_rl_repo")

import numpy as np
import ml_dtypes

import concourse.bacc as bacc
import concourse.mybir as mybir
import concourse.tile as tile
from concourse.bass_utils import run_bass_kernel_spmd
from concourse.tile import add_dep_helper

B, L, FD, G = 256, 512, 64, 32
H1, H2, DH = 512, 512, 256
NCORES = 8
BL = B // NCORES          # 32 samples per core
NBLK = BL // 4            # 8 blocks of 4 samples

f32 = mybir.dt.float32
bf16 = mybir.dt.bfloat16
f8e4 = mybir.dt.float8e4
AF = mybir.ActivationFunctionType
ALU = mybir.AluOpType
AX = mybir.AxisListType
DR = mybir.MatmulPerfMode.DoubleRow


def _build():
    nc = bacc.Bacc("TRN2", target_bir_lowering=False, debug=False,
                   num_devices=NCORES)

    d = {}
    d["xlt"] = nc.dram_tensor("xlt", [128, NBLK * 1024], bf16, kind="ExternalInput").ap()
    d["xgt"] = nc.dram_tensor("xgt", [G, BL], f32, kind="ExternalInput").ap()
    d["w1s"] = nc.dram_tensor("w1s", [128, H1], bf16, kind="ExternalInput").ap()
    d["w2f8"] = nc.dram_tensor("w2f8", [128, 2048], f8e4, kind="ExternalInput").ap()
    d["wawt"] = nc.dram_tensor("wawt", [128, 8], bf16, kind="ExternalInput").ap()
    d["wd1g"] = nc.dram_tensor("wd1g", [G, DH], f32, kind="ExternalInput").ap()
    d["wd1p"] = nc.dram_tensor("wd1p", [128, 4 * DH], f32, kind="ExternalInput").ap()
    d["wd2"] = nc.dram_tensor("wd2", [128, 2], f32, kind="ExternalInput").ap()
    d["b1"] = nc.dram_tensor("b1", [128, 4], f32, kind="ExternalInput").ap()
    d["b2"] = nc.dram_tensor("b2", [128, 4], f32, kind="ExternalInput").ap()
    d["bd1"] = nc.dram_tensor("bd1", [128, 2], f32, kind="ExternalInput").ap()
    d["scal"] = nc.dram_tensor("scal", [128, 4], f32, kind="ExternalInput").ap()
    out_d = nc.dram_tensor("out", [2, BL // 2], f32, kind="ExternalOutput").ap()

    with tile.TileContext(nc) as tc:
        with (
            tc.tile_pool(name="wp", bufs=1) as wp,
            tc.tile_pool(name="pp", bufs=1) as pp,
            tc.tile_pool(name="xp", bufs=4) as xp,
            tc.tile_pool(name="h1p", bufs=2) as h1p,
            tc.tile_pool(name="p2sb", bufs=1) as p2sb,
            tc.tile_pool(name="awp", bufs=4) as awp,
            tc.tile_pool(name="psp", bufs=1, space="PSUM") as psp,
        ):
            w1s_t = wp.tile([128, H1], bf16)
            w2f8_t = wp.tile([128, 2048], f8e4)
            wawt_t = wp.tile([128, 8], bf16)
            xgt_t = wp.tile([G, BL], f32)
            wd1g_t = wp.tile([G, DH], f32)
            wd1p_t = wp.tile([128, 4 * DH], f32)
            wd2_t = wp.tile([128, 2], f32)
            b1_t = wp.tile([128, 4], f32)
            b2_t = wp.tile([128, 4], f32)
            bd1_t = wp.tile([128, 2], f32)
            scal_t = wp.tile([128, 4], f32)

            # scratch + dummy gelu first: the gelu ACT_TABLE_LOAD runs
            # concurrently with the input DMAs.  The dummy reads and writes
            # disjoint cols of its own scratch (the write allocates the
            # tile) so the wu_sb memset / warm-ups need not wait for it.
            wu_sb = wp.tile([128, 128], f32, name="wu_sb")
            scr_t = wp.tile([1, 4], f32, name="scr_t")
            # tail tiles: engine APs must start at a 32-aligned partition,
            # so the absorb/atten pre-acts live in one [128, L] tile with
            # four aligned 16-row groups: abs(h) at rows 64h, att(h) at
            # rows 64h+32 (h = sample half); the gaps stay unused.
            aw_all = p2sb.tile([128, L], f32, name="aw_all")
            texpx = p2sb.tile([128, L], f32, name="texpx")
            outT = p2sb.tile([64, BL], f32, name="outT")
            outTT = p2sb.tile([64, BL], f32, name="outTT")

            # startup DMAs: first-needed tensors first, spread across the
            # three DGE queues (sync / scalar-hwdge / gpsimd-swdge)
            x2_pre = {}

            def fetch_x2(g):
                xt = xp.tile([128, 2 * L], bf16, name=f"x2_{g}", tag="x2")
                nc.sync.dma_start(xt[:], d["xlt"][:, g * 1024:(g + 1) * 1024])
                x2_pre[g] = xt

            fetch_x2(0)
            nc.scalar.dma_start(w1s_t[:], d["w1s"][:])
            nc.scalar.dma_start(b1_t[:], d["b1"][:])
            fetch_x2(1)
            fetch_x2(2)
            nc.scalar.activation(scr_t[0:1, 0:1], scr_t[0:1, 1:2], AF.Gelu)
            nc.gpsimd.memset(wu_sb[:], 0.0)
            nc.gpsimd.memset(texpx[:, L - 1:L], 1.0)
            nc.gpsimd.memset(outT[:], 0.0)
            for nm, t in [("w2f8", w2f8_t), ("b2", b2_t), ("wawt", wawt_t),
                          ("xgt", xgt_t), ("wd1g", wd1g_t), ("wd1p", wd1p_t),
                          ("wd2", wd2_t), ("bd1", bd1_t), ("scal", scal_t)]:
                nc.gpsimd.dma_start(t[:], d[nm][:])

            pooled_t = pp.tile([128, 4 * BL], f32)   # [h_part, mc*32 + s]
            h2all = pp.tile([128, NBLK * 4 * 2048], bf16)  # [feat, (g*4+mc)*2048 + j*512 + l]

            # all of PSUM as one tile; pairs P0=[0:2048], P1=[2048:4096]
            ps_all = psp.tile([128, 4096], f32)

            # PE warm-up: back-to-back matmuls on scratch data so the HAM
            # clock gate reaches K=8/8 before (and until) the real work.
            for i in range(8):
                nc.tensor.matmul(ps_all[:, 3968:4096], wu_sb[:], wu_sb[:],
                                 start=True, stop=True)

            h1tiles = {}
            unit = 0  # ACT-unit counter; parity picks the PSUM pair

            def filler(n):
                # keep the PE activity monitor (HAM) above its clock-gate
                # threshold; writes land in bank 7's tail, which the next
                # pair-1 unit overwrites with start=True (ordering-only).
                for i in range(n):
                    nc.tensor.matmul(ps_all[:, 3968:4096], wu_sb[:],
                                     wu_sb[:], start=True, stop=True)

            def l1_unit(g, mc):
                """One layer-1 ACT unit: 4 row-tiled bf16 MM pairs + gelu->fp8."""
                nonlocal unit
                x2t = x2_pre[g]
                h1t = h1tiles[g]
                pbase = (unit % 2) * 2048
                for h in range(2):
                    for sl in range(2):
                        nc.tensor.matmul(
                            ps_all[:, pbase + (2 * h + sl) * 512:
                                   pbase + (2 * h + sl) * 512 + 512],
                            w1s_t[64 * sl:64 * sl + 64, mc * 128:(mc + 1) * 128],
                            x2t[64 * sl:64 * sl + 64, h * 512:(h + 1) * 512],
                            start=True, stop=True)
                nc.scalar.activation(
                    h1t[:, mc * 2048:(mc + 1) * 2048],
                    ps_all[:, pbase:pbase + 2048],
                    AF.Gelu, bias=b1_t[:, mc:mc + 1])
                unit += 1

            def emit_reduce(g, mc):
                # per-sample pooling: sum over L on the DVE
                h2base = (g * 4 + mc) * 2048
                nc.vector.reduce_sum(
                    pooled_t[:, mc * BL + g * 4:mc * BL + g * 4 + 4],
                    h2all[:, h2base:h2base + 2048].rearrange(
                        "p (j n) -> p j n", j=4),
                    axis=AX.X)

            def l2_unit(g, mc, reduce=True):
                """One layer-2 ACT unit: 8 fp8 DoubleRow MMs + gelu + pooling."""
                nonlocal unit
                h1t = h1tiles[g]
                pbase = (unit % 2) * 2048
                for kcp in range(2):
                    wk = w2f8_t[:, kcp * 1024:(kcp + 1) * 1024].rearrange(
                        "p (ko mcm) -> p ko mcm", ko=2)
                    hk = h1t[:, (2 * kcp) * 2048:(2 * kcp + 2) * 2048].rearrange(
                        "p (ko n) -> p ko n", ko=2)
                    for j in range(4):
                        nc.tensor.matmul(
                            ps_all[:, pbase + j * 512:pbase + (j + 1) * 512],
                            wk[:, :, mc * 128:(mc + 1) * 128],
                            hk[:, :, j * 512:(j + 1) * 512],
                            start=(kcp == 0), stop=(kcp == 1),
                            perf_mode=DR)
                h2base = (g * 4 + mc) * 2048
                nc.scalar.activation(
                    h2all[:, h2base:h2base + 2048],
                    ps_all[:, pbase:pbase + 2048],
                    AF.Gelu, bias=b2_t[:, mc:mc + 1])
                unit += 1
                if reduce:
                    emit_reduce(g, mc)

            def burst_mm(b, col):
                # absorb/atten pre-acts for block b: col-tiled burst into
                # psum cols [col, col+512), 4 samples in 32-col PE strips
                for c in range(4):
                    for j in range(4):
                        nc.tensor.matmul(
                            ps_all[32 * j:32 * j + 2, col:col + 512],
                            wawt_t[:, 2 * c:2 * c + 2],
                            h2all[:, (b * 4 + c) * 2048 + j * 512:
                                  (b * 4 + c) * 2048 + (j + 1) * 512],
                            start=(c == 0), stop=(c == 3),
                            tile_position=(0, 32 * j))

            def burst_drain(b, col, eng):
                aw_sb = awp.tile([128, L], f32, name=f"aw_sb_{b}", tag="aw_sb")
                if eng == "scalar":
                    nc.scalar.copy(aw_sb[:], ps_all[:, col:col + 512])
                else:
                    nc.vector.tensor_copy(aw_sb[:], ps_all[:, col:col + 512])
                h, r = b // 4, (b % 4) * 4
                nc.sync.dma_start(aw_all[64 * h + r:64 * h + r + 4, :],
                                  aw_sb[0:128:32, :])
                nc.sync.dma_start(aw_all[64 * h + 32 + r:64 * h + 32 + r + 4, :],
                                  aw_sb[1:128:32, :])

            # prologue: block 0's layer 1, with fillers (before the first
            # pair-1 reader only -- later filler slots would FIFO-block the
            # 0.1us-slack l2 matmul chains) to keep HAM from throttling.
            h1tiles[0] = h1p.tile([128, 4 * 2048], f8e4, name="h1t_0", tag="h1t")
            l1_unit(0, 0)
            filler(12)
            for mc in range(1, 4):
                l1_unit(0, mc)

            # main loop, software-pipelined: block g+1's layer-1 units
            # interleave with block g's layer-2 units.  The loop is kept
            # pristine: every l2 matmul chain (1.9us) must start the
            # instant the previous l2 gelu drains pair 1 (0.1us slack), so
            # no extra PE work may ride inside the loop.
            for g in range(NBLK):
                if g + 3 < NBLK:
                    fetch_x2(g + 3)
                if g + 1 < NBLK:
                    h1tiles[g + 1] = h1p.tile([128, 4 * 2048], f8e4,
                                              name=f"h1t_{g+1}", tag="h1t")
                for mc in range(4):
                    if g + 1 < NBLK:
                        l1_unit(g + 1, mc)
                    l2_unit(g, mc)

            # ---- phase 2 ----
            # Burst schedule: pair-0 banks free 2us before the stream ends
            # (second-to-last gelu), pair-1 banks at the end.  b0/b1 pre-run
            # on banks 1/2; b7 (whose data is only ready at the very end)
            # gets bank 0 and the first post-stream PE slot so half 1's aw
            # rows start draining immediately; the rest follow.  The dense
            # head rides in bank 7 between b3 and b4 so its gelus + the
            # table switch complete while the late bursts still drain.
            # NOTE: each drain is emitted IMMEDIATELY after its burst --
            # later writers of the same psum columns (d1in, dps) would
            # otherwise become program-order RAW sources for the copy.
            burst_mm(0, 0)         # banks 0-3 free once gelu62 drains pair 0
            burst_drain(0, 0, "scalar")
            burst_mm(1, 512)
            burst_drain(1, 512, "scalar")
            burst_mm(2, 1024)
            burst_drain(2, 1024, "scalar")
            burst_mm(3, 1536)
            burst_drain(3, 1536, "dve")
            burst_mm(7, 3584)      # bank 7; first pair-1 slot after gelu63
            burst_drain(7, 3584, "scalar")

            d1in = [ps_all[:, 0:BL], ps_all[:, 512:512 + BL]]
            for lo, hi in ((0, 28), (28, 32)):
                # Xg part + pooled blocks 0-6 accumulate as soon as banks
                # 0/1 drain (sample cols 0:28); only block 7's pooled
                # columns (28:32) wait for the last pooling reduce.
                for mc2 in range(2):
                    ps = d1in[mc2][:, lo:hi]
                    nc.tensor.matmul(ps, wd1g_t[:, mc2 * 128:(mc2 + 1) * 128],
                                     xgt_t[:, lo:hi], start=True, stop=False)
                    for hc in range(4):
                        nc.tensor.matmul(
                            ps,
                            wd1p_t[:, hc * DH + mc2 * 128:hc * DH + (mc2 + 1) * 128],
                            pooled_t[:, hc * BL + lo:hc * BL + hi],
                            start=False, stop=(hc == 3))

            burst_mm(4, 2048)      # banks 4-6, after the dense-head input
            burst_drain(4, 2048, "dve")
            burst_mm(5, 2560)
            burst_drain(5, 2560, "dve")
            burst_mm(6, 3072)
            burst_drain(6, 3072, "dve")

            d1t = []
            gelu_insts = []
            for mc2 in range(2):
                t = p2sb.tile([128, BL], f32, name=f"d1t_{mc2}")
                gi = nc.scalar.activation(t[:], d1in[mc2], AF.Gelu,
                                          bias=bd1_t[:, mc2:mc2 + 1])
                gelu_insts.append(gi)
                d1t.append(t)

            # single table switch to the ln/exp set, after the last gelu
            from concourse.hw_specs import get_activation_tables
            tabs = get_activation_tables(nc.m.arch)
            set_id = next(i for i, fns in enumerate(tabs.values())
                          if AF.Exp in fns and AF.Ln in fns)
            li = nc.scalar.add_instruction(mybir.InstLoadActFuncSet(
                name=nc.scalar.bass.get_next_instruction_name(),
                act_func_set_id=set_id, ins=[], outs=[]))
            for gi in gelu_insts:
                add_dep_helper(li.ins, gi.ins, sync=True,
                               reason="ACT table set order: gelu before ln/exp")

            # per-half wd2 contraction so each half's d starts at partition 0
            dps_h = [ps_all[0:16, 1024:1025], ps_all[0:16, 1088:1089]]
            for h in range(2):
                nc.tensor.matmul(dps_h[h], d1t[0][:, 16 * h:16 * h + 16],
                                 wd2_t[:, 0:1], start=True, stop=False)
                nc.tensor.matmul(dps_h[h], d1t[1][:, 16 * h:16 * h + 16],
                                 wd2_t[:, 1:2], start=False, stop=True)

            # tail, one pass over the [128, L] aw tile (all 32 samples):
            # softplus via ln(1+exp), T via exp(-cumsum) of the reversed
            # atten scan; garbage rows compute for free.
            e_aw = p2sb.tile([128, L], f32, name="e_aw")
            sp_t = p2sb.tile([128, L], f32, name="sp_t")
            incl = p2sb.tile([128, L], f32, name="incl")
            contrib = p2sb.tile([128, L], f32, name="contrib")
            cap = p2sb.tile([128, 1], f32, name="cap")
            # per-half softplus so half 0 (drained first) starts while the
            # late bursts are still landing
            ei = nc.scalar.activation(e_aw[0:64, :], aw_all[0:64, :], AF.Exp,
                                      bias=scal_t[0:64, 0:1])
            add_dep_helper(ei.ins, li.ins, sync=True,
                           reason="ACT table set order: ln set before exp")
            nc.scalar.activation(sp_t[0:64, :], e_aw[0:64, :], AF.Ln, bias=1.0)
            nc.scalar.activation(e_aw[64:128, :], aw_all[64:128, :], AF.Exp,
                                 bias=scal_t[64:128, 0:1])
            nc.scalar.activation(sp_t[64:128, :], e_aw[64:128, :], AF.Ln,
                                 bias=1.0)
            # incl[l] = sum_{l'>=l} softplus(atten): add-scan over reversed
            # L, shifting the atten rows down 32 partitions so T lands on
            # the same partitions as the absorb rows (the STT below needs
            # both SBUF inputs at one base partition; >32-partition
            # patterns must start at partition 0, hence per-half scans).
            for h in range(2):
                spt_rev = sp_t[64 * h + 32:64 * h + 48, L - 1::-1]
                incl_rev = incl[64 * h:64 * h + 16, L - 1::-1]
                nc.vector.tensor_tensor_scan(incl_rev, spt_rev, spt_rev, 0.0,
                                             ALU.add, ALU.bypass)
                # texpx[l] = T[l] = exp(-incl[l+1]); col L-1 pre-set to 1.0
                nc.scalar.activation(texpx[64 * h:64 * h + 16, 0:L - 1],
                                     incl[64 * h:64 * h + 16, 1:L],
                                     AF.Exp, scale=-1.0)
            for h in range(2):
                nc.vector.scalar_tensor_tensor(
                    contrib[64 * h:64 * h + 16, :],
                    sp_t[64 * h:64 * h + 16, :], 1.0,
                    texpx[64 * h:64 * h + 16, :], ALU.mult, ALU.mult,
                    accum_out=cap[64 * h:64 * h + 16, 0:1])
            for h in range(2):
                # out = (cap + bd2) + d, into rows 32h:32h+16 of outT col 0
                nc.vector.scalar_tensor_tensor(
                    outT[32 * h:32 * h + 16, 0:1],
                    cap[64 * h:64 * h + 16, 0:1],
                    scal_t[64 * h:64 * h + 16, 2:3], dps_h[h],
                    ALU.add, ALU.add)
            # transpose so the output DMA is two contiguous descriptors
            nc.vector.transpose(outTT[:], outT[:])
            nc.sync.dma_start(out_d[:], outTT[0:64:32, 0:16])

    nc.compile()
    return nc


_CACHE = {}


def _prep_inputs(inputs):
    f = lambda a: np.ascontiguousarray(np.asarray(a, dtype=np.float32))
    Xg, Xl = f(inputs["Xg"]), f(inputs["Xl"])
    W1, b1 = f(inputs["W1"]), f(inputs["b1"])
    W2, b2 = f(inputs["W2"]), f(inputs["b2"])
    wa, ba = f(inputs["wa"]), f(inputs["ba"])
    wt, bt = f(inputs["wt"]), f(inputs["bt"])
    Wd1, bd1 = f(inputs["Wd1"]), f(inputs["bd1"])
    Wd2, bd2 = f(inputs["Wd2"]), f(inputs["bd2"])

    shared = {
        "w1s": np.ascontiguousarray(np.concatenate([W1, W1], axis=0)).astype(ml_dtypes.bfloat16),
        # [k, kc', ko, mc, m]: W2 row = kc'*256 + ko*128 + k, col = mc*128 + m
        "w2f8": np.ascontiguousarray(
            W2.reshape(2, 2, 128, 4, 128).transpose(2, 0, 1, 3, 4)
            .reshape(128, 2048)).astype(ml_dtypes.float8_e4m3),
        "wawt": np.ascontiguousarray(
            np.concatenate([wa, wt], axis=1).reshape(4, 128, 2)
            .transpose(1, 0, 2).reshape(128, 8)).astype(ml_dtypes.bfloat16),
        "wd1g": np.ascontiguousarray(Wd1[:G]),
        "wd1p": np.ascontiguousarray(
            (Wd1[G:] / np.float32(L)).reshape(4, 128, DH)
            .transpose(1, 0, 2).reshape(128, 4 * DH)),
        "wd2": np.ascontiguousarray(Wd2.reshape(2, 128).T),
        "b1": np.ascontiguousarray(b1.reshape(4, 128).T),
        "b2": np.ascontiguousarray(b2.reshape(4, 128).T),
        "bd1": np.ascontiguousarray(bd1.reshape(2, 128).T),
    }
    scal = np.zeros((128, 4), np.float32)
    for h in range(2):
        scal[64 * h:64 * h + 16, 0] = ba.reshape(-1)[0]
        scal[64 * h + 32:64 * h + 48, 0] = bt.reshape(-1)[0]
    scal[:, 2] = bd2.reshape(-1)[0]
    shared["scal"] = scal

    in_maps = []
    for c in range(NCORES):
        s = slice(c * BL, (c + 1) * BL)
        m = dict(shared)
        # [sl*64+f, g*1024 + h*512 + l]: 2KB-contiguous per partition row
        # per block so each block's fetch is 128 descriptors, not 256
        m["xlt"] = np.ascontiguousarray(
            Xl[s].reshape(NBLK, 2, 2, L, FD).transpose(2, 4, 0, 1, 3)
            .reshape(128, NBLK * 1024)).astype(ml_dtypes.bfloat16)
        m["xgt"] = np.ascontiguousarray(Xg[s].T)
        in_maps.append(m)
    return in_maps


def _run(inputs, trace=False, tmpdir=None):
    if "nc" not in _CACHE:
        _CACHE["nc"] = _build()
    nc = _CACHE["nc"]
    in_maps = _prep_inputs(inputs)
    res = run_bass_kernel_spmd(nc, in_maps, list(range(NCORES)),
                               trace=trace, tmpdir=tmpdir)
    out = np.concatenate([res.results[c]["out"].reshape(BL, 1)
                          for c in range(NCORES)], axis=0)
    return out.astype(np.float32), res


def kernel(**inputs) -> np.ndarray:
    out, _ = _run(inputs)
    return out


# revision 33
# speedup vs baseline: 1.1614x; 1.0225x over previous
"""Trainium2 Bass kernel for nn_CumulativeShadeRegressor.

Model (per sample): per-leaf MLP encoder [L, FD] -> [L, H2] (two gelu
layers), softplus absorb/atten heads, a top-to-bottom exponential
transmittance scan over L, mean-pooling over L, and a small dense head on
[Xg | pooled].

Strategy: data-parallel over B across 8 NeuronCores (32 samples/core).
The ACT (scalar) engine is the bottleneck: 64 gelu ACTIVATEs of 2048 cols
each (~2us apiece) form a ~126us stream that everything else must hide
under.  The kernel therefore:
  * layer 1 (K=64) runs as row-tiled bf16 matmul pairs (2 concurrent MMs
    in disjoint 64-row PE strips);
  * layer 2 runs in fp8e4 DoubleRow mode (2 MACs/cell, contraction 256
    per pass) with h1 quantized to fp8 by the gelu ACT itself;
  * gelu ACT ops are batched to N=2048 (4 samples per instruction, PSUM
    pair ring of 2x4 banks) to amortize the ~300-cycle ACT init;
  * startup DMAs are spread across all three DGE queues (sync, scalar,
    gpsimd) so the first x2 block + w1s land ~3us earlier;
  * PE warm-up/filler matmuls keep the HAM clock gate at K=8/8 through
    the l1-only prologue (a K=4/8 dip used to stall the ACT stream 4.6us);
  * phase 2 gives each absorb/atten burst its own PSUM bank (no ping-pong
    chains), drains 4 bursts on the scalar engine and 4 on the DVE, and
    runs the softplus/scan/transmittance tail in two pipelined halves
    (samples 0-15 and 16-31) so scalar, DVE, and DMA latency overlap;
  * the final [32,1] result is transposed to one SBUF row so the output
    DMA is a single descriptor (the 32-descriptor version left its
    completion semaphore trickling for ~5us after the data landed).
"""
import sys

sys.path.insert(0, "/opt/trn_rl_repo")

import numpy as np
import ml_dtypes

import concourse.bacc as bacc
import concourse.mybir as mybir
import concourse.tile as tile
from concourse.bass_utils import run_bass_kernel_spmd
from concourse.tile import add_dep_helper

B, L, FD, G = 256, 512, 64, 32
H1, H2, DH = 512, 512, 256
NCORES = 8
BL = B // NCORES          # 32 samples per core
NBLK = BL // 4            # 8 blocks of 4 samples

f32 = mybir.dt.float32
bf16 = mybir.dt.bfloat16
f8e4 = mybir.dt.float8e4
AF = mybir.ActivationFunctionType
ALU = mybir.AluOpType
AX = mybir.AxisListType
DR = mybir.MatmulPerfMode.DoubleRow


def _build():
    nc = bacc.Bacc("TRN2", target_bir_lowering=False, debug=False,
                   num_devices=NCORES)

    d = {}
    d["xlt"] = nc.dram_tensor("xlt", [128, NBLK * 1024], bf16, kind="ExternalInput").ap()
    d["xgt"] = nc.dram_tensor("xgt", [G, BL], f32, kind="ExternalInput").ap()
    d["w1s"] = nc.dram_tensor("w1s", [128, H1], bf16, kind="ExternalInput").ap()
    d["w2f8"] = nc.dram_tensor("w2f8", [128, 2048], f8e4, kind="ExternalInput").ap()
    d["wawt"] = nc.dram_tensor("wawt", [128, 8], bf16, kind="ExternalInput").ap()
    d["wd1g"] = nc.dram_tensor("wd1g", [G, DH], f32, kind="ExternalInput").ap()
    d["wd1p"] = nc.dram_tensor("wd1p", [128, 4 * DH], f32, kind="ExternalInput").ap()
    d["wd2"] = nc.dram_tensor("wd2", [128, 2], f32, kind="ExternalInput").ap()
    d["b1"] = nc.dram_tensor("b1", [128, 4], f32, kind="ExternalInput").ap()
    d["b2"] = nc.dram_tensor("b2", [128, 4], f32, kind="ExternalInput").ap()
    d["bd1"] = nc.dram_tensor("bd1", [128, 2], f32, kind="ExternalInput").ap()
    d["scal"] = nc.dram_tensor("scal", [128, 4], f32, kind="ExternalInput").ap()
    out_d = nc.dram_tensor("out", [2, BL // 2], f32, kind="ExternalOutput").ap()

    with tile.TileContext(nc) as tc:
        with (
            tc.tile_pool(name="wp", bufs=1) as wp,
            tc.tile_pool(name="pp", bufs=1) as pp,
            tc.tile_pool(name="xp", bufs=4) as xp,
            tc.tile_pool(name="h1p", bufs=2) as h1p,
            tc.tile_pool(name="p2sb", bufs=1) as p2sb,
            tc.tile_pool(name="awp", bufs=1) as awp,
            tc.tile_pool(name="psp", bufs=1, space="PSUM") as psp,
        ):
            w1s_t = wp.tile([128, H1], bf16)
            w2f8_t = wp.tile([128, 2048], f8e4)
            wawt_t = wp.tile([128, 8], bf16)
            xgt_t = wp.tile([G, BL], f32)
            wd1g_t = wp.tile([G, DH], f32)
            wd1p_t = wp.tile([128, 4 * DH], f32)
            wd2_t = wp.tile([128, 2], f32)
            b1_t = wp.tile([128, 4], f32)
            b2_t = wp.tile([128, 4], f32)
            bd1_t = wp.tile([128, 2], f32)
            scal_t = wp.tile([128, 4], f32)

            # scratch + dummy gelu first: the gelu ACT_TABLE_LOAD runs
            # concurrently with the input DMAs.  The dummy reads and writes
            # disjoint cols of its own scratch (the write allocates the
            # tile) so the wu_sb memset / warm-ups need not wait for it.
            wu_sb = wp.tile([128, 128], f32, name="wu_sb")
            scr_t = wp.tile([1, 4], f32, name="scr_t")
            # tail tiles: engine APs must start at a 32-aligned partition,
            # so the absorb/atten pre-acts live in one [128, L] tile with
            # four aligned 16-row groups: abs(h) at rows 64h, att(h) at
            # rows 64h+32 (h = sample half); the gaps stay unused.
            aw_all = p2sb.tile([128, L], f32, name="aw_all")
            texpx = p2sb.tile([128, L], f32, name="texpx")
            outT = p2sb.tile([64, BL], f32, name="outT")
            outTT = p2sb.tile([64, BL], f32, name="outTT")

            # startup DMAs: first-needed tensors first, spread across the
            # three DGE queues (sync / scalar-hwdge / gpsimd-swdge)
            x2_pre = {}

            def fetch_x2(g):
                xt = xp.tile([128, 2 * L], bf16, name=f"x2_{g}", tag="x2")
                nc.sync.dma_start(xt[:], d["xlt"][:, g * 1024:(g + 1) * 1024])
                x2_pre[g] = xt

            fetch_x2(0)
            nc.scalar.dma_start(w1s_t[:], d["w1s"][:])
            nc.scalar.dma_start(b1_t[:], d["b1"][:])
            nc.scalar.dma_start(b2_t[:], d["b2"][:])
            nc.scalar.dma_start(w2f8_t[:], d["w2f8"][:])
            fetch_x2(1)
            fetch_x2(2)
            nc.scalar.activation(scr_t[0:1, 0:1], scr_t[0:1, 1:2], AF.Gelu)
            nc.gpsimd.memset(wu_sb[:], 0.0)
            nc.gpsimd.memset(texpx[:, L - 1:L], 1.0)
            nc.gpsimd.memset(outT[:], 0.0)
            for nm, t in [("wawt", wawt_t), ("xgt", xgt_t), ("wd1g", wd1g_t),
                          ("wd1p", wd1p_t), ("wd2", wd2_t), ("bd1", bd1_t),
                          ("scal", scal_t)]:
                nc.gpsimd.dma_start(t[:], d[nm][:])

            pooled_t = pp.tile([128, 4 * BL], f32)   # [h_part, mc*32 + s]
            h2all = pp.tile([128, NBLK * 4 * 2048], bf16)  # [feat, (g*4+mc)*2048 + j*512 + l]

            # all of PSUM as one tile; pairs P0=[0:2048], P1=[2048:4096]
            ps_all = psp.tile([128, 4096], f32)

            # PE warm-up: back-to-back matmuls on scratch data so the HAM
            # clock gate reaches K=8/8 before (and until) the real work.
            for i in range(8):
                nc.tensor.matmul(ps_all[:, 3968:4096], wu_sb[:], wu_sb[:],
                                 start=True, stop=True)

            h1tiles = {}
            unit = 0  # ACT-unit counter; parity picks the PSUM pair

            def filler(n):
                # keep the PE activity monitor (HAM) above its clock-gate
                # threshold; writes land in bank 7's tail, which the next
                # pair-1 unit overwrites with start=True (ordering-only).
                for i in range(n):
                    nc.tensor.matmul(ps_all[:, 3968:4096], wu_sb[:],
                                     wu_sb[:], start=True, stop=True)

            def l1_unit(g, mc):
                """One layer-1 ACT unit: 4 row-tiled bf16 MM pairs + gelu->fp8."""
                nonlocal unit
                x2t = x2_pre[g]
                h1t = h1tiles[g]
                pbase = (unit % 2) * 2048
                for h in range(2):
                    for sl in range(2):
                        nc.tensor.matmul(
                            ps_all[:, pbase + (2 * h + sl) * 512:
                                   pbase + (2 * h + sl) * 512 + 512],
                            w1s_t[64 * sl:64 * sl + 64, mc * 128:(mc + 1) * 128],
                            x2t[64 * sl:64 * sl + 64, h * 512:(h + 1) * 512],
                            start=True, stop=True)
                nc.scalar.activation(
                    h1t[:, mc * 2048:(mc + 1) * 2048],
                    ps_all[:, pbase:pbase + 2048],
                    AF.Gelu, bias=b1_t[:, mc:mc + 1])
                unit += 1

            def emit_reduce(g, mc):
                # per-sample pooling: sum over L on the DVE
                h2base = (g * 4 + mc) * 2048
                nc.vector.reduce_sum(
                    pooled_t[:, mc * BL + g * 4:mc * BL + g * 4 + 4],
                    h2all[:, h2base:h2base + 2048].rearrange(
                        "p (j n) -> p j n", j=4),
                    axis=AX.X)

            def l2_unit(g, mc, reduce=True):
                """One layer-2 ACT unit: 8 fp8 DoubleRow MMs + gelu + pooling."""
                nonlocal unit
                h1t = h1tiles[g]
                pbase = (unit % 2) * 2048
                for kcp in range(2):
                    wk = w2f8_t[:, kcp * 1024:(kcp + 1) * 1024].rearrange(
                        "p (ko mcm) -> p ko mcm", ko=2)
                    hk = h1t[:, (2 * kcp) * 2048:(2 * kcp + 2) * 2048].rearrange(
                        "p (ko n) -> p ko n", ko=2)
                    for j in range(4):
                        nc.tensor.matmul(
                            ps_all[:, pbase + j * 512:pbase + (j + 1) * 512],
                            wk[:, :, mc * 128:(mc + 1) * 128],
                            hk[:, :, j * 512:(j + 1) * 512],
                            start=(kcp == 0), stop=(kcp == 1),
                            perf_mode=DR)
                h2base = (g * 4 + mc) * 2048
                nc.scalar.activation(
                    h2all[:, h2base:h2base + 2048],
                    ps_all[:, pbase:pbase + 2048],
                    AF.Gelu, bias=b2_t[:, mc:mc + 1])
                unit += 1
                if reduce:
                    emit_reduce(g, mc)

            def burst_mm(b, col):
                # absorb/atten pre-acts for block b: col-tiled burst into
                # psum cols [col, col+512), 4 samples in 32-col PE strips
                for c in range(4):
                    for j in range(4):
                        nc.tensor.matmul(
                            ps_all[32 * j:32 * j + 2, col:col + 512],
                            wawt_t[:, 2 * c:2 * c + 2],
                            h2all[:, (b * 4 + c) * 2048 + j * 512:
                                  (b * 4 + c) * 2048 + (j + 1) * 512],
                            start=(c == 0), stop=(c == 3),
                            tile_position=(0, 32 * j))

            def burst_drain(b, col, eng):
                # one aw_sb buffer per burst (tag-ring reuse would couple
                # late copies to earlier bursts' DMA completions); the
                # DVE-drained bursts compact via the gpsimd queue so the 16
                # aw DMAs don't serialize on one issue queue.
                aw_sb = awp.tile([128, L], f32, name=f"aw_sb_{b}",
                                 tag=f"aw_sb_{b}")
                if eng == "scalar":
                    nc.scalar.copy(aw_sb[:], ps_all[:, col:col + 512])
                    dma = nc.sync.dma_start
                else:
                    nc.vector.tensor_copy(aw_sb[:], ps_all[:, col:col + 512])
                    dma = nc.gpsimd.dma_start
                h, r = b // 4, (b % 4) * 4
                dma(aw_all[64 * h + r:64 * h + r + 4, :], aw_sb[0:128:32, :])
                dma(aw_all[64 * h + 32 + r:64 * h + 32 + r + 4, :],
                    aw_sb[1:128:32, :])

            # prologue: block 0's layer 1, with fillers (before the first
            # pair-1 reader only -- later filler slots would FIFO-block the
            # 0.1us-slack l2 matmul chains) to keep HAM from throttling.
            h1tiles[0] = h1p.tile([128, 4 * 2048], f8e4, name="h1t_0", tag="h1t")
            l1_unit(0, 0)
            filler(12)
            for mc in range(1, 4):
                l1_unit(0, mc)

            # main loop, software-pipelined: block g+1's layer-1 units
            # interleave with block g's layer-2 units.  The loop is kept
            # pristine: every l2 matmul chain (1.9us) must start the
            # instant the previous l2 gelu drains pair 1 (0.1us slack), so
            # no extra PE work may ride inside the loop.
            for g in range(NBLK):
                if g + 3 < NBLK:
                    fetch_x2(g + 3)
                if g + 1 < NBLK:
                    h1tiles[g + 1] = h1p.tile([128, 4 * 2048], f8e4,
                                              name=f"h1t_{g+1}", tag="h1t")
                for mc in range(4):
                    if g + 1 < NBLK:
                        l1_unit(g + 1, mc)
                    l2_unit(g, mc)

            # ---- phase 2 ----
            # Burst schedule: pair-0 banks free 2us before the stream ends
            # (second-to-last gelu), pair-1 banks at the end.  b0/b1 pre-run
            # on banks 1/2; b7 (whose data is only ready at the very end)
            # gets bank 0 and the first post-stream PE slot so half 1's aw
            # rows start draining immediately; the rest follow.  The dense
            # head rides in bank 7 between b3 and b4 so its gelus + the
            # table switch complete while the late bursts still drain.
            # NOTE: each drain is emitted IMMEDIATELY after its burst --
            # later writers of the same psum columns (d1in, dps) would
            # otherwise become program-order RAW sources for the copy.
            burst_mm(0, 0)         # banks 0-3 free once gelu62 drains pair 0
            burst_drain(0, 0, "scalar")
            burst_mm(1, 512)
            burst_drain(1, 512, "scalar")
            burst_mm(2, 1024)
            burst_drain(2, 1024, "scalar")
            burst_mm(3, 1536)
            burst_drain(3, 1536, "dve")
            burst_mm(7, 3584)      # bank 7; first pair-1 slot after gelu63
            burst_drain(7, 3584, "scalar")
            burst_mm(4, 2048)      # banks 4-6 right behind it
            burst_drain(4, 2048, "dve")
            burst_mm(5, 2560)
            burst_drain(5, 2560, "dve")
            burst_mm(6, 3072)
            burst_drain(6, 3072, "dve")

            d1in = [ps_all[:, 0:BL], ps_all[:, 512:512 + BL]]
            for lo, hi in ((0, 28), (28, 32)):
                # Xg part + pooled blocks 0-6 accumulate once banks 0/1
                # drain (sample cols 0:28); only block 7's pooled columns
                # (28:32) wait for the last pooling reduce.
                for mc2 in range(2):
                    ps = d1in[mc2][:, lo:hi]
                    nc.tensor.matmul(ps, wd1g_t[:, mc2 * 128:(mc2 + 1) * 128],
                                     xgt_t[:, lo:hi], start=True, stop=False)
                    for hc in range(4):
                        nc.tensor.matmul(
                            ps,
                            wd1p_t[:, hc * DH + mc2 * 128:hc * DH + (mc2 + 1) * 128],
                            pooled_t[:, hc * BL + lo:hc * BL + hi],
                            start=False, stop=(hc == 3))

            d1t = []
            gelu_insts = []
            for mc2 in range(2):
                t = p2sb.tile([128, BL], f32, name=f"d1t_{mc2}")
                gi = nc.scalar.activation(t[:], d1in[mc2], AF.Gelu,
                                          bias=bd1_t[:, mc2:mc2 + 1])
                gelu_insts.append(gi)
                d1t.append(t)

            # single table switch to the ln/exp set, after the last gelu
            from concourse.hw_specs import get_activation_tables
            tabs = get_activation_tables(nc.m.arch)
            set_id = next(i for i, fns in enumerate(tabs.values())
                          if AF.Exp in fns and AF.Ln in fns)
            li = nc.scalar.add_instruction(mybir.InstLoadActFuncSet(
                name=nc.scalar.bass.get_next_instruction_name(),
                act_func_set_id=set_id, ins=[], outs=[]))
            for gi in gelu_insts:
                add_dep_helper(li.ins, gi.ins, sync=True,
                               reason="ACT table set order: gelu before ln/exp")

            # per-half wd2 contraction so each half's d starts at partition 0
            dps_h = [ps_all[0:16, 1024:1025], ps_all[0:16, 1088:1089]]
            for h in range(2):
                nc.tensor.matmul(dps_h[h], d1t[0][:, 16 * h:16 * h + 16],
                                 wd2_t[:, 0:1], start=True, stop=False)
                nc.tensor.matmul(dps_h[h], d1t[1][:, 16 * h:16 * h + 16],
                                 wd2_t[:, 1:2], start=False, stop=True)
            # keep the PE busy through the drain/scan phase so HAM doesn't
            # halve the clock mid-tail (writes land in drained bank-0 cols
            # that nothing reads again)
            for i in range(10):
                nc.tensor.matmul(ps_all[:, 256:384], wu_sb[:], wu_sb[:],
                                 start=True, stop=True)

            # tail, one pass over the [128, L] aw tile (all 32 samples):
            # softplus via ln(1+exp), T via exp(-cumsum) of the reversed
            # atten scan; garbage rows compute for free.
            e_aw = p2sb.tile([128, L], f32, name="e_aw")
            sp_t = p2sb.tile([128, L], f32, name="sp_t")
            incl = p2sb.tile([128, L], f32, name="incl")
            contrib = p2sb.tile([128, L], f32, name="contrib")
            cap = p2sb.tile([128, 1], f32, name="cap")
            # per-half softplus so half 0 (drained first) starts while the
            # late bursts are still landing
            ei = nc.scalar.activation(e_aw[0:64, :], aw_all[0:64, :], AF.Exp,
                                      bias=scal_t[0:64, 0:1])
            add_dep_helper(ei.ins, li.ins, sync=True,
                           reason="ACT table set order: ln set before exp")
            nc.scalar.activation(sp_t[0:64, :], e_aw[0:64, :], AF.Ln, bias=1.0)
            nc.scalar.activation(e_aw[64:128, :], aw_all[64:128, :], AF.Exp,
                                 bias=scal_t[64:128, 0:1])
            nc.scalar.activation(sp_t[64:128, :], e_aw[64:128, :], AF.Ln,
                                 bias=1.0)
            # incl[l] = sum_{l'>=l} softplus(atten): add-scan over reversed
            # L, shifting the atten rows down 32 partitions so T lands on
            # the same partitions as the absorb rows (the STT below needs
            # both SBUF inputs at one base partition; >32-partition
            # patterns must start at partition 0, hence per-half scans).
            for h in range(2):
                spt_rev = sp_t[64 * h + 32:64 * h + 48, L - 1::-1]
                incl_rev = incl[64 * h:64 * h + 16, L - 1::-1]
                nc.vector.tensor_tensor_scan(incl_rev, spt_rev, spt_rev, 0.0,
                                             ALU.add, ALU.bypass)
                # texpx[l] = T[l] = exp(-incl[l+1]); col L-1 pre-set to 1.0
                nc.scalar.activation(texpx[64 * h:64 * h + 16, 0:L - 1],
                                     incl[64 * h:64 * h + 16, 1:L],
                                     AF.Exp, scale=-1.0)
            for h in range(2):
                nc.vector.scalar_tensor_tensor(
                    contrib[64 * h:64 * h + 16, :],
                    sp_t[64 * h:64 * h + 16, :], 1.0,
                    texpx[64 * h:64 * h + 16, :], ALU.mult, ALU.mult,
                    accum_out=cap[64 * h:64 * h + 16, 0:1])
            for h in range(2):
                # out = (cap + bd2) + d, into rows 32h:32h+16 of outT col 0
                nc.vector.scalar_tensor_tensor(
                    outT[32 * h:32 * h + 16, 0:1],
                    cap[64 * h:64 * h + 16, 0:1],
                    scal_t[64 * h:64 * h + 16, 2:3], dps_h[h],
                    ALU.add, ALU.add)
            # transpose so the output DMA is two contiguous descriptors
            nc.vector.transpose(outTT[:], outT[:])
            nc.sync.dma_start(out_d[:], outTT[0:64:32, 0:16])

    nc.compile()
    return nc


_CACHE = {}


def _prep_inputs(inputs):
    f = lambda a: np.ascontiguousarray(np.asarray(a, dtype=np.float32))
    Xg, Xl = f(inputs["Xg"]), f(inputs["Xl"])
    W1, b1 = f(inputs["W1"]), f(inputs["b1"])
    W2, b2 = f(inputs["W2"]), f(inputs["b2"])
    wa, ba = f(inputs["wa"]), f(inputs["ba"])
    wt, bt = f(inputs["wt"]), f(inputs["bt"])
    Wd1, bd1 = f(inputs["Wd1"]), f(inputs["bd1"])
    Wd2, bd2 = f(inputs["Wd2"]), f(inputs["bd2"])

    shared = {
        "w1s": np.ascontiguousarray(np.concatenate([W1, W1], axis=0)).astype(ml_dtypes.bfloat16),
        # [k, kc', ko, mc, m]: W2 row = kc'*256 + ko*128 + k, col = mc*128 + m
        "w2f8": np.ascontiguousarray(
            W2.reshape(2, 2, 128, 4, 128).transpose(2, 0, 1, 3, 4)
            .reshape(128, 2048)).astype(ml_dtypes.float8_e4m3),
        "wawt": np.ascontiguousarray(
            np.concatenate([wa, wt], axis=1).reshape(4, 128, 2)
            .transpose(1, 0, 2).reshape(128, 8)).astype(ml_dtypes.bfloat16),
        "wd1g": np.ascontiguousarray(Wd1[:G]),
        "wd1p": np.ascontiguousarray(
            (Wd1[G:] / np.float32(L)).reshape(4, 128, DH)
            .transpose(1, 0, 2).reshape(128, 4 * DH)),
        "wd2": np.ascontiguousarray(Wd2.reshape(2, 128).T),
        "b1": np.ascontiguousarray(b1.reshape(4, 128).T),
        "b2": np.ascontiguousarray(b2.reshape(4, 128).T),
        "bd1": np.ascontiguousarray(bd1.reshape(2, 128).T),
    }
    scal = np.zeros((128, 4), np.float32)
    for h in range(2):
        scal[64 * h:64 * h + 16, 0] = ba.reshape(-1)[0]
        scal[64 * h + 32:64 * h + 48, 0] = bt.reshape(-1)[0]
    scal[:, 2] = bd2.reshape(-1)[0]
    shared["scal"] = scal

    in_maps = []
    for c in range(NCORES):
        s = slice(c * BL, (c + 1) * BL)
        m = dict(shared)
        # [sl*64+f, g*1024 + h*512 + l]: 2KB-contiguous per partition row
        # per block so each block's fetch is 128 descriptors, not 256
        m["xlt"] = np.ascontiguousarray(
            Xl[s].reshape(NBLK, 2, 2, L, FD).transpose(2, 4, 0, 1, 3)
            .reshape(128, NBLK * 1024)).astype(ml_dtypes.bfloat16)
        m["xgt"] = np.ascontiguousarray(Xg[s].T)
        in_maps.append(m)
    return in_maps


def _run(inputs, trace=False, tmpdir=None):
    if "nc" not in _CACHE:
        _CACHE["nc"] = _build()
    nc = _CACHE["nc"]
    in_maps = _prep_inputs(inputs)
    res = run_bass_kernel_spmd(nc, in_maps, list(range(NCORES)),
                               trace=trace, tmpdir=tmpdir)
    out = np.concatenate([res.results[c]["out"].reshape(BL, 1)
                          for c in range(NCORES)], axis=0)
    return out.astype(np.float32), res


def kernel(**inputs) -> np.ndarray:
    out, _ = _run(inputs)
    return out
